# revision 25
# baseline (speedup 1.0000x reference)
"""Trainium2 Bass kernel for the CTRF dense_cnn problem.

y[b,t,o] = b[o] + sum_{lag in [-10,40]} sum_d W[o,(lag+10)*64+d] * x[b,t-lag,d]

Strategy (8 NeuronCores, data-parallel over batch, 2 batches/core):
  - Pair adjacent timesteps: z[k,u] = x[2u + k//64, k%64] in R^128, so each
    K=128 matmul covers two lags at once (full PE array instead of K=64).
  - The 51 lags become 52 stationary [128,128] matrices M_j = [Wblk_j; Wblk_{j-1}];
    even j feed even output timesteps, odd j odd timesteps, each a 26-tap conv
    over u. Taps are ordered descending-j so tap k reads zt cols [u0+k, u0+k+W).
  - All layout transforms (x->z pairing, W->M_j, output de-interleave) are done
    on the host, so the device does nothing but DMA, matmul, and bias-add.
  - bf16 operands/outputs (fp32 PSUM accumulate): halves DMA bytes and speeds
    LDWEIGHTS; well within the 2e-2 tolerance.
  - First/last groups run 256 cols wide: the first matmul gates on ~160KB of
    DMA instead of ~360KB, and the final drain (act+store) pipelines behind
    the last 26 matmuls.
"""

import os
import sys

os.environ.setdefault("MYCRO_LOCAL_CACHE", "1")

for _p in ("/opt/trn_rl_repo", "/root/.axon_site/_ro/trn_rl_repo"):
    if os.path.isdir(_p) and _p not in sys.path:
        sys.path.insert(0, _p)

import ml_dtypes
import numpy as np

import concourse.bass as bass  # noqa: E402
import concourse.mybir as mybir  # noqa: E402
import concourse.tile as tile  # noqa: E402
from concourse import bacc  # noqa: E402
from concourse.bass import ts  # noqa: E402
from concourse.bass_utils import run_bass_kernel_spmd  # noqa: E402

N_CORES = 8
B, T, D, O = 16, 2048, 64, 128
NLAGS = 51
U = T // 2          # pair rows per batch
BPC = B // N_CORES  # batches per core
NJ = NLAGS + 1      # stationary matrices
PAD_L = 20          # zero cols left of z (tap 0 at u0=0 reads col 0)
PAD_R = 12          # zero cols right of z (need >= 5)
ZCOLS = PAD_L + U + PAD_R
NWARM = 17          # HAM warmup matmuls (256 cols each); sized to cover the
                    # p90 input-DMA arrival (~10.9us) — a PE idle gap >1us
                    # between warmup and the first real matmul resets the
                    # HAM clock ramp, which costs ~3us, while overshooting
                    # costs only ~0.2us per extra warmup.

BF16 = ml_dtypes.bfloat16

# Weight chunks per parity, js in descending order (consumption order).
WP_CHUNKS = [[7, 7, 6, 6], [13, 13]]


def _js_desc(par):
    return list(range(par, NJ, 2))[::-1]


def _cum(par):
    out = [0]
    for n in WP_CHUNKS[par]:
        out.append(out[-1] + n)
    return out


def _chunk_for(par, k):
    cum = _cum(par)
    base = 0 if par == 0 else len(WP_CHUNKS[0])
    for c in range(len(WP_CHUNKS[par])):
        if k < cum[c + 1]:
            return base + c, k - cum[c]
    raise ValueError(k)


_WP_SIZES = WP_CHUNKS[0] + WP_CHUNKS[1]


def _build():
    nc = bacc.Bacc(
        "TRN2", target_bir_lowering=False, debug=False, num_devices=N_CORES
    )
    f32 = mybir.dt.float32
    bf16 = mybir.dt.bfloat16

    z_d = nc.declare_dram_parameter("z", [BPC, 128, U], bf16, isOutput=False)
    wp_ds = [
        nc.declare_dram_parameter(f"wp{c}", [128, n, O], bf16, isOutput=False)
        for c, n in enumerate(_WP_SIZES)
    ]
    b_d = nc.declare_dram_parameter("bvec", [O, 1], f32, isOutput=False)
    y_d = nc.declare_dram_parameter("y", [BPC, 2, O, U], bf16, isOutput=True)

    with tile.TileContext(nc) as tc:
        with (
            tc.tile_pool(name="consts", bufs=1) as consts,
            tc.tile_pool(name="zt", bufs=2) as zt_pool,
            tc.tile_pool(name="osb", bufs=4) as osb_pool,
            tc.tile_pool(name="osbq", bufs=2) as osbq_pool,
            tc.tile_pool(name="pacc", bufs=5, space="PSUM") as pacc_pool,
            tc.tile_pool(name="ptail", bufs=2, space="PSUM") as ptail_pool,
            tc.tile_pool(name="warm", bufs=1, space="PSUM") as warm_pool,
        ):
            # HAM warmup: the PE clock-gate opens (0.65/1.2 -> 2.4 GHz) only
            # after ~5.5us of sustained matmul activity; burn the input-DMA
            # wait (~3.2us until the first chunks land) on matmuls that
            # depend only on a DVE memset.
            wsrc = consts.tile([128, 256], bf16, tag="wsrc")
            nc.vector.memset(wsrc[:], 1.0)
            warm_ps = warm_pool.tile([128, 256], f32, tag="warm")
            for _ in range(NWARM):
                nc.tensor.matmul(
                    warm_ps[:], wsrc[:, 0:128], wsrc[:], start=True, stop=True
                )

            zt0 = zt_pool.tile([128, ZCOLS], bf16, tag="zt")
            zt1 = zt_pool.tile([128, ZCOLS], bf16, tag="zt")
            for zt in (zt0, zt1):
                nc.vector.memset(zt[:, 0:PAD_L], 0.0)
                nc.vector.memset(zt[:, PAD_L + U :], 0.0)

            bias_sb = consts.tile([O, 1], f32)
            wp_sbs = [
                consts.tile(
                    [128, _WP_SIZES[c], O], bf16,
                    name=f"wp{c}", tag=f"wp{c}",
                )
                for c in range(len(_WP_SIZES))
            ]

            # DMA triggers issue from two queues in parallel: z + bias on the
            # Scalar HWDGE, weights on Sync (stores later also on Sync).
            # Per-queue program order = consumption order.
            def dma_z(zt, bb, c0, c1):
                nc.scalar.dma_start(
                    zt[:, PAD_L + c0 : PAD_L + c1], z_d[bb][:, c0:c1]
                )

            dma_z(zt0, 0, 0, 512)
            nc.sync.dma_start(wp_sbs[0][:], wp_ds[0][:])
            dma_z(zt0, 0, 512, U)
            nc.sync.dma_start(wp_sbs[1][:], wp_ds[1][:])
            nc.scalar.dma_start(bias_sb[:], b_d[:])
            nc.sync.dma_start(wp_sbs[2][:], wp_ds[2][:])
            nc.sync.dma_start(wp_sbs[3][:], wp_ds[3][:])
            nc.sync.dma_start(wp_sbs[4][:], wp_ds[4][:])
            nc.sync.dma_start(wp_sbs[5][:], wp_ds[5][:])
            nc.sync.dma_start(zt1[:, PAD_L : PAD_L + 512], z_d[1][:, 0:512])
            nc.sync.dma_start(zt1[:, PAD_L + 512 : PAD_L + U], z_d[1][:, 512:U])

            def emit_group(bb, zt, par, u0, width):
                if width == 512:
                    pacc = pacc_pool.tile([128, 512], f32)
                else:
                    pacc = ptail_pool.tile([128, 256], f32)
                for k in range(26):
                    c, off = _chunk_for(par, k)
                    nc.tensor.matmul(
                        pacc[:],
                        wp_sbs[c][:, off, :],
                        zt[:, u0 + k : u0 + k + width],
                        start=(k == 0),
                        stop=(k == 25),
                    )
                yv = y_d[bb][par]
                if width == 512:
                    osb = osb_pool.tile([128, 512], bf16)
                else:
                    osb = osbq_pool.tile([128, 256], bf16)
                nc.scalar.activation(
                    osb[:],
                    pacc[:],
                    mybir.ActivationFunctionType.Identity,
                    bias=bias_sb[:],
                )
                nc.sync.dma_start(yv[:, u0 : u0 + width], osb[:])

            # Last logical 512-col group runs as 2x256 so the final
            # act+store drains pipelined behind the last 26 matmuls.
            groups = [
                (0, zt0, 0, 0, 512), (0, zt0, 0, 512, 512),
                (0, zt0, 1, 0, 512), (0, zt0, 1, 512, 512),
                (1, zt1, 0, 0, 512), (1, zt1, 0, 512, 512),
                (1, zt1, 1, 0, 512),
                (1, zt1, 1, 512, 256), (1, zt1, 1, 768, 256),
            ]
            for g in groups:
                emit_group(*g)
    nc.compile()
    return nc


_NC_CACHE = {}


def _get_program():
    if "nc" not in _NC_CACHE:
        _NC_CACHE["nc"] = _build()
    return _NC_CACHE["nc"]


def _prep_inputs(x, W, b):
    x = np.ascontiguousarray(x, dtype=np.float32)
    W = np.ascontiguousarray(W, dtype=np.float32)
    b = np.ascontiguousarray(b, dtype=np.float32)
    # z[b, k, u] = x[b, 2u + k//64, k%64]
    z = np.ascontiguousarray(
        x.reshape(B, U, 2, D).transpose(0, 2, 3, 1).reshape(B, 128, U)
    ).astype(BF16)
    Wt = W.reshape(O, NLAGS, D).transpose(1, 2, 0)  # [j, d, o]
    wp = np.zeros((NJ, 128, O), dtype=np.float32)
    wp[:NLAGS, :D, :] = Wt
    wp[1:, D:, :] = Wt
    bvec = np.ascontiguousarray(b.reshape(O, 1))
    wp_chunks = {}
    ci = 0
    for par in range(2):
        js = _js_desc(par)
        cum = _cum(par)
        for c in range(len(WP_CHUNKS[par])):
            sel = js[cum[c] : cum[c + 1]]
            wp_chunks[f"wp{ci}"] = np.ascontiguousarray(
                wp[sel].transpose(1, 0, 2)
            ).astype(BF16)
            ci += 1
    maps = []
    for c in range(N_CORES):
        m = {"z": z[c * BPC : (c + 1) * BPC], "bvec": bvec}
        m.update(wp_chunks)
        maps.append(m)
    return maps


def _post(res):
    # y param [BPC, 2, O, U] -> [BPC, T, O] with t = 2u + par
    outs = []
    for c in range(N_CORES):
        yp = np.asarray(res.results[c]["y"])
        outs.append(
            yp.transpose(0, 3, 1, 2).reshape(BPC, T, O).astype(np.float32)
        )
    return np.concatenate(outs, axis=0)


def kernel(x, W, b):
    in_maps = _prep_inputs(x, W, b)
    res = run_bass_kernel_spmd(
        _get_program(), in_maps, core_ids=list(range(N_CORES))
    )
    return _post(res)


def _ensure_ntff_hook():
    """The agent image's antenv lacks axon_hooks, so run_bass_kernel_spmd's
    trace path degrades to no-profile. Seed an equivalent module backed by
    the ctypes NTFF profiler from trn_agent_boot."""
    try:
        from antenv.axon_hooks import get_axon_ntff_profile_hook

        if get_axon_ntff_profile_hook() is not None:
            return True
    except ImportError:
        pass
    try:
        import types

        site_dir = "/root/.axon_site"
        if site_dir not in sys.path and os.path.isdir(site_dir):
            sys.path.insert(0, site_dir)
        from trn_agent_boot.trn_boot import _ntff_profile_via_ctypes

        hook = _ntff_profile_via_ctypes("/opt/axon/libaxon_pjrt.so")
        if hook is None:
            return False
        mod = types.ModuleType("antenv.axon_hooks")
        mod.get_axon_ntff_profile_hook = lambda: hook
        mod.set_axon_ntff_profile_hook = lambda h: None
        sys.modules["antenv.axon_hooks"] = mod
        import antenv

        antenv.axon_hooks = mod
        return True
    except Exception:
        return False


def kernel_traced(x, W, b, **kwargs):
    """Like kernel() but requests an NTFF trace; returns (y, BassKernelResults).

    Dev-loop only (test.py); the graded kernel() path never traces. The
    artifact upload is stubbed out since this container has no bucket access.
    """
    _ensure_ntff_hook()
    from concourse import bass_utils as _bu

    in_maps = _prep_inputs(x, W, b)
    orig_upload = _bu.upload_artifacts
    _bu.upload_artifacts = lambda tmpdir: f"local:{tmpdir}"
    try:
        res = run_bass_kernel_spmd(
            _get_program(), in_maps, core_ids=list(range(N_CORES)), trace=True, **kwargs
        )
    finally:
        _bu.upload_artifacts = orig_upload
    y = _post(res)
    return y, res
